# revision 14
# baseline (speedup 1.0000x reference)
"""Trainium2 Bass kernel for the triangle-network CustomLoss problem.

probs[x,y,z] = sum_{i,j,k} Pa[i] Pb[j] Pc[k] A[j,k,x] B[k,i,y] C[i,j,z]
d = sum(y_true * (log(y_true+1e-10) - log(clip(probs, 1e-10, 1.0))))

K = 512 (i/j/k cardinality), O = 8 (x/y/z outputs).

Sharding: 4-way over i x 2-way over j (8 cores).  Each core:
  stage 1 (TensorE): T[i, j, y, x] = sum_k (B*Pc)[k,y,i]^T @ A[k,(j,x)]
      as 128x [K=128, M=128, N=512] bf16 matmuls accumulated in PSUM.
  stage 2 (TensorE): partial[z,(y,x)] += Cw[i,(j,z)]^T @ T[i,(j,y,x)]
      as 256x [K=128, M=8, N=64] matmuls accumulated in one PSUM tile,
      where Cw = C * Pa[i] * Pb[j] is folded on DVE.
Host: gather 8 partial [8,64] prob tensors, sum (the contraction
unshard), reorder to [x,y,z], and apply the tiny KL epilogue.
"""

import numpy as np
import ml_dtypes

K = 512
O = 8
NS = 3
IP = 4  # i-parts
JP = 2  # j-parts
ISL = K // IP  # 128
JSL = K // JP  # 256
N_CORES = 8

_CACHE = {}


def _build_bass():
    import concourse.bass as bass
    import concourse.mybir as mybir
    import concourse.tile as tile

    bf16 = mybir.dt.bfloat16
    f32 = mybir.dt.float32

    KC_ = 4
    nc = bass.Bass()
    # Packed inputs: one dma_start each => one completion semaphore each,
    # because the DVE TensorScalar/TensorTensor ISA structs carry at most ONE
    # sync-wait.  All DVE-consumed data (pc, pa, b, c, pbz) arrives via the
    # sync-engine HWDGE ring; the matmul moving operand (a) via the
    # scalar-engine HWDGE ring so both stream concurrently.
    #   w1 cols: [0:4]=pc  [4:5]=pa  [8:8+4096]=b (4 k-chunks x 1024)
    #   w3 cols: [0:2048]=c  [2048:4096]=pbz
    W1C = 8 + 4 * (O * ISL)
    a_d = nc.dram_tensor("a_t", [128, KC_ * (JSL * O)], bf16, kind="ExternalInput")
    w1_d = nc.dram_tensor("w1", [128, W1C], bf16, kind="ExternalInput")
    w3_d = nc.dram_tensor("w3", [ISL, 2 * JSL * O], bf16, kind="ExternalInput")
    out_d = nc.dram_tensor("out", [O, O * O], f32, kind="ExternalOutput")

    KC = KC_  # k chunks of 128
    NB = 4  # stage-1 moving chunks of 512 = 64 j's each

    with tile.TileContext(nc) as tc:
        with (
            tc.tile_pool(name="inp", bufs=1) as inp,
            tc.tile_pool(name="work", bufs=1) as work,
            tc.tile_pool(name="ps1", bufs=1, space="PSUM") as ps1,
            tc.tile_pool(name="ps2", bufs=1, space="PSUM") as ps2,
        ):
            w1_sb = inp.tile([128, W1C], bf16, name="w1_sb")
            nc.sync.dma_start(out=w1_sb, in_=w1_d[:, :])
            a_sb = inp.tile([128, KC * (JSL * O)], bf16, name="a_sb")
            nc.scalar.dma_start(out=a_sb, in_=a_d[:, :])
            w3_sb = inp.tile([ISL, 2 * JSL * O], bf16, name="w3_sb")
            nc.sync.dma_start(out=w3_sb, in_=w3_d[:, :])

            # pc/pa bounced through DVE-local f32 copies; after the first
            # copy waits on w1's semaphore, all later DVE ops chain through
            # DVE program order (single DVE sem).
            pc2 = work.tile([128, 4], f32, name="pc2")
            nc.vector.tensor_copy(pc2, w1_sb[:, 0:4])
            pa2 = work.tile([ISL, 1], f32, name="pa2")
            nc.vector.tensor_copy(pa2, w1_sb[:, 4:5])

            bw = []
            for c in range(KC):
                bwt = work.tile([128, O * ISL], bf16, name=f"bw{c}", tag=f"bw{c}")
                # Bw[k, (y,i)] = B[k, (y,i)] * Pc[k]
                nc.vector.tensor_scalar_mul(
                    bwt, w1_sb[:, 8 + c * O * ISL : 8 + (c + 1) * O * ISL],
                    pc2[:, c : c + 1],
                )
                bw.append(bwt)

            # Cw = (C * Pb[j]) * Pa[i]; the tensor_tensor goes first because
            # both of its inputs ride w3's single DMA semaphore (1 wait), and
            # the per-partition Pa scaling then chains through DVE order.
            cw1 = work.tile([ISL, JSL * O], bf16, name="cw1")
            nc.vector.tensor_mul(
                cw1, w3_sb[:, 0 : JSL * O], w3_sb[:, JSL * O : 2 * JSL * O]
            )
            cw = work.tile([ISL, JSL * O], bf16, name="cw")
            nc.vector.tensor_scalar_mul(cw, cw1, pa2[:, 0:1])

            # T[i, (j, y, x)] bf16
            t_sb = work.tile([ISL, JSL * O * O], bf16, name="t_sb")
            t_view = t_sb.rearrange("p (j y x) -> p j y x", j=JSL, y=O, x=O)

            # stage 1.  Fixed psum slots with deterministic reuse (group t
            # reuses slot t % NPS, last used by group t - NPS).  Before each
            # reusing group, a 1-column junk ldweights reads the t-NPS drain
            # region: it absorbs the drain-done (ACT) wait so the matmul
            # itself carries only the single PE wait the 1-slot MM ISA
            # struct allows.
            NPS = 6
            ps_tiles = [
                ps1.tile([128, 512], f32, name=f"ps{s}", tag=f"ps{s}")
                for s in range(NPS)
            ]
            for y in range(O):
                for n in range(NB):
                    t = y * NB + n
                    if t >= NPS:
                        yp, np_ = divmod(t - NPS, NB)
                        col = np_ * 64 * O * O + yp * O
                        nc.tensor.ldweights(weights=t_sb[:, col : col + 1])
                    ps = ps_tiles[t % NPS]
                    for c in range(KC):
                        nc.tensor.matmul(
                            ps,
                            lhsT=bw[c][:, y * ISL : (y + 1) * ISL],
                            rhs=a_sb[
                                :, c * (JSL * O) + n * 512 : c * (JSL * O) + (n + 1) * 512
                            ],
                            start=(c == 0),
                            stop=(c == KC - 1),
                        )
                    # drain [i, (j64, x8)] -> T[i, j=n*64+jl, y, x]
                    nc.scalar.copy(
                        out=t_view[:, n * 64 : (n + 1) * 64, y, :],
                        in_=ps.rearrange("p (j x) -> p j x", j=64, x=O),
                    )

            # stage 2: partial[z, (y,x)] += Cw_j^T @ T_j over all j.
            # Junk ldweights on the last-drained column absorbs the ACT wait
            # for the whole T tile.
            nc.tensor.ldweights(
                weights=t_sb[:, JSL * O * O - 1 : JSL * O * O]
            )
            pso = ps2.tile([O, O * O], f32, name="pso")
            for j in range(JSL):
                nc.tensor.matmul(
                    pso,
                    lhsT=cw[:, j * O : (j + 1) * O],
                    rhs=t_sb[:, j * O * O : (j + 1) * O * O],
                    start=(j == 0),
                    stop=(j == JSL - 1),
                )
            out_sb = work.tile([O, O * O], f32, name="out_sb")
            nc.scalar.copy(out=out_sb, in_=pso)
            nc.sync.dma_start(out=out_d[:, :], in_=out_sb)

    _strip_redundant_self_waits(nc)
    return nc


def _strip_redundant_self_waits(nc):
    """Drop sync-waits that are provably redundant, via a transitive
    happens-before (vector clock over semaphore values) analysis.

    Most TRN2 compute ISA structs carry a single sync-wait slot and the SP
    drain only a few; Tile's sem assignment is per-proc minimal but not
    transitively minimal, so it emits waits already implied through other
    waits (e.g. the tail drain re-waiting every DMA sem the compute engines
    already observed) or through same-engine program order.  walrus hard-fails
    on those instructions.

    Model: instructions are processed in Tile's emitted order (a valid
    execution order).  Each instruction's dispatch-known state = join of the
    engine's previous dispatch state, the completion state of the
    same-engine instruction TWO back (in-order engines; the margin covers
    dispatch-ahead overlap of the immediately preceding op), and the
    threshold snapshots of its kept waits.  A wait (S >= v) is stripped iff
    the dispatch-known state already has S >= v.  Completion events (sem
    increments) snapshot the dispatch state plus own updates; waits on
    semaphores that are ever decremented (barrier machinery) are never
    stripped."""
    f = nc.m.functions[0]

    def join(a, b):
        for k, v in b.items():
            if a.get(k, 0) < v:
                a[k] = v
        return a

    INC_MODES = ("sem-inc", "sem-add-imm")

    # first pass: find sems with any non-inc update (barriers) -> untouchable
    dirty = set()
    for blk in f.blocks:
        for inst in blk.instructions:
            si = inst.sync_info
            if si:
                for u in si.on_update:
                    if u.update_mode not in INC_MODES:
                        dirty.add(u.ant_name)

    sem_cum = {}
    sem_events = {}  # sem -> list of (cum_after, prefix_joined_snapshot)
    disp = {}  # engine -> dispatch-known snapshot {sem: value}
    hist = {}  # engine -> list of completion snapshots
    n_stripped = 0
    for blk in f.blocks:
        for inst in blk.instructions:
            e = inst.engine
            si = inst.sync_info
            base = dict(disp.get(e, {}))
            eh = hist.setdefault(e, [])
            if len(eh) >= 2:
                join(base, eh[-2])
            if si is not None and si.on_wait:
                kept = []
                for w in si.on_wait:
                    s = w.ant_name
                    if (
                        w.wait_mode == "sem-ge-imm"
                        and s not in dirty
                        and base.get(s, 0) >= w.wait_value
                    ):
                        n_stripped += 1
                        continue
                    kept.append(w)
                    if w.wait_mode == "sem-ge-imm" and s not in dirty:
                        # learn the threshold snapshot
                        evs = sem_events.get(s)
                        if evs is not None:
                            for cum_after, snap in evs:
                                if cum_after >= w.wait_value:
                                    join(base, snap)
                                    break
                if len(kept) != len(si.on_wait):
                    si.on_wait = kept
            disp[e] = base
            # Async completions (DMA data-landed sems) are only learnable by
            # explicitly waiting on the sem, never via engine program order.
            is_async = type(inst).__name__ == "InstDMACopy"
            comp = dict(base)
            if si is not None:
                for u in si.on_update:
                    if u.update_mode in INC_MODES and u.ant_name not in dirty:
                        s = u.ant_name
                        sem_cum[s] = sem_cum.get(s, 0) + u.update_value
                        ev_state = dict(comp)
                        ev_state[s] = max(ev_state.get(s, 0), sem_cum[s])
                        if not is_async:
                            comp[s] = ev_state[s]
                        evs = sem_events.setdefault(s, [])
                        prefix = dict(evs[-1][1]) if evs else {}
                        join(prefix, ev_state)
                        evs.append((sem_cum[s], prefix))
            eh.append(comp)
    return n_stripped


def _prep_core_inputs(y_pred):
    """Slice/transpose the flat y_pred into per-core input maps (layout only,
    plus bf16 cast; all arithmetic happens on device)."""
    bf = ml_dtypes.bfloat16
    srcs = y_pred[0, : NS * K].reshape(NS, K)
    pf = y_pred[0, NS * K :]
    sz = K * K * O
    A = pf[:sz].reshape(K, K, O)  # [j, k, x]
    B = pf[sz : 2 * sz].reshape(K, K, O)  # [k, i, y]
    C = pf[2 * sz :].reshape(K, K, O)  # [i, j, z]
    Pa, Pb, Pc = srcs[0], srcs[1], srcs[2]

    pc_host = np.ascontiguousarray(Pc.reshape(4, 128).T).astype(bf)  # [128, 4]

    in_maps = []
    for core in range(N_CORES):
        ip, jp = core % IP, core // IP
        isl = slice(ip * ISL, (ip + 1) * ISL)
        jsl = slice(jp * JSL, (jp + 1) * JSL)
        # a: [k, j, x] -> packed [128, 4 * 2048] (k-chunk c at cols c*2048..)
        a_t = np.ascontiguousarray(A[jsl].transpose(1, 0, 2)).astype(bf)
        a_p = np.ascontiguousarray(
            a_t.reshape(4, 128, JSL * O).transpose(1, 0, 2)
        ).reshape(128, 4 * JSL * O)
        # b: [k, y, i] -> packed [128, 4 * 1024]
        b_t = np.ascontiguousarray(B[:, isl, :].transpose(0, 2, 1)).astype(bf)
        b_p = np.ascontiguousarray(
            b_t.reshape(4, 128, O * ISL).transpose(1, 0, 2)
        ).reshape(128, 4 * O * ISL)
        # w1 = [pc(4) pa(1) pad(3) b(4096)]
        w1 = np.zeros((128, 8 + 4 * O * ISL), dtype=bf)
        w1[:, 0:4] = pc_host
        w1[:, 4] = Pa[isl].astype(bf)
        w1[:, 8:] = b_p
        # w3 = [c(2048) pbz(2048)]
        c_t = np.ascontiguousarray(C[isl, jsl]).astype(bf).reshape(ISL, JSL * O)
        pbz = np.broadcast_to(
            np.repeat(Pb[jsl], O).astype(bf)[None, :], (ISL, JSL * O)
        )
        w3 = np.concatenate([c_t, pbz], axis=1)
        in_maps.append({"a_t": a_p, "w1": w1, "w3": np.ascontiguousarray(w3)})
    return in_maps


def _run(in_maps, trace=False, trace_kwargs=None):
    from concourse.bass_utils import run_bass_kernel_spmd

    if "nc" not in _CACHE:
        _CACHE["nc"] = _build_bass()
    kwargs = {}
    if trace:
        kwargs = dict(trace=True, trace_cores=list(range(N_CORES)))
        if trace_kwargs:
            kwargs.update(trace_kwargs)
    res = run_bass_kernel_spmd(
        _CACHE["nc"], in_maps, core_ids=list(range(N_CORES)), **kwargs
    )
    return res


def kernel(y_pred, y_true, _trace=False):
    y_pred = np.asarray(y_pred, dtype=np.float32)
    y_true = np.asarray(y_true, dtype=np.float32)
    in_maps = _prep_core_inputs(y_pred)
    res = _run(in_maps, trace=_trace)
    # gather/unshard: contraction sharding -> sum of partials
    S = np.zeros((O, O * O), dtype=np.float64)
    for r in res.results:
        S += r["out"].astype(np.float64)
    probs = (
        S.reshape(O, O, O).transpose(2, 1, 0).reshape(-1).astype(np.float32)
    )  # [z,y,x] -> [x,y,z]
    logp = np.log(np.clip(probs, 1e-10, 1.0)).astype(np.float32)
    d = np.float32(np.sum(y_true * (np.log(y_true + 1e-10) - logp)))
    if _trace:
        kernel._last_result = res
    return d, probs
